# revision 1
# baseline (speedup 1.0000x reference)
"""Causal GQA attention for Trainium2, sharded across 8 NeuronCores.

Problem: q [2, 2048, 32, 128], k/v [2, 2048, 8, 128] fp32, causal, GQA
group = 4. Sharding: core i gets kv-head i plus its 4 q-heads, both
batch elements (tensor-parallel over heads, no collectives); outputs
are concatenated on the head axis.

Per-core kernel (flash-style, transposed-S layout), one global
software pipeline over (head, q-tile, k-chunk-pair):
  - inputs are converted to bf16 on the HOST (halves input DMA bytes
    and removes all on-device dtype-conversion copies; bf16 QK costs
    ~3e-3 rel error total).
  - S^T[k,q] = (QK^T)^T per 128-wide k-chunk via f32-PSUM matmuls, with
    the causal column range truncated per chunk (no widening: bf16 has
    no narrow-column penalty).
  - S chunks are emitted as PSUM pair tiles [128,2,512]; ONE ScalarE
    exp per pair (halves ACT call count) writes P^T pairs in bf16, with
    a constant bias folded in (softmax-invariant).
  - causal masking is post-exp on the otherwise-idle GPSIMD engine
    (affine_select triangles + memset of fully-masked columns), off the
    exp critical path.
  - O^T and the denominator accumulate in PSUM via bf16 matmuls per
    chunk at each chunk's own causal column base; the denominator uses
    an all-ones stationary so the result lands pre-broadcast.
  - normalization: DVE reciprocal straight from PSUM + one multiply;
    output stores are issued per q-tile from the SP queue.
O/D trail the S/exp stream by PIPE_PAIRS pairs globally (across q-tile
and head boundaries), with double-buffered O/D PSUM accumulators, so no
per-tile pipeline drain occurs.
"""

import math

import ml_dtypes
import numpy as np

import concourse.tile as tile
from concourse import bacc, mybir
from concourse.bass_utils import run_bass_kernel_spmd

P = 128
F32 = mybir.dt.float32
MMDT = mybir.dt.bfloat16
BF16 = mybir.dt.bfloat16
MASK_VAL = -1e6
PIPE_PAIRS = 2
# shift exp into fp8e4m3 range: softmax is invariant to a constant bias;
# max observed scale*S is ~6.2 and e^6.2 > 448 (fp8 max) -> inf
EXP_BIAS = -2.5


def emit_attention(nc, tc, ctx, q_ap, k_ap, v_ap, o_ap, B, QL, KL, HL, D):
    assert D == P
    QT = 512
    KC = P
    n_qt = QL // QT
    n_kc_total = KL // KC
    qt_per_kc = QT // KC
    scale = 1.0 / math.sqrt(D)

    sb = ctx.enter_context(tc.tile_pool(name="sb", bufs=1))
    sb_q = ctx.enter_context(tc.tile_pool(name="sb_q", bufs=2))
    sb_pt = ctx.enter_context(tc.tile_pool(name="sb_pt", bufs=5))
    sb_o = ctx.enter_context(tc.tile_pool(name="sb_o", bufs=2))
    ps_s = ctx.enter_context(tc.tile_pool(name="ps_s", bufs=2, space="PSUM"))
    ps_o = ctx.enter_context(tc.tile_pool(name="ps_o", bufs=2, space="PSUM"))
    ps_d = ctx.enter_context(tc.tile_pool(name="ps_d", bufs=2, space="PSUM"))

    # --- constants / engine warmups (keep Pool OFF the critical path) ---
    warm = sb.tile([P, 1], F32, name="warm")
    nc.vector.memset(warm[:], 0.0)
    nc.scalar.activation(warm[:], warm[:],
                         mybir.ActivationFunctionType.Exp)
    poolwarm = sb.tile([P, 8], F32, name="poolwarm")
    nc.gpsimd.memset(poolwarm[:], 0.0)
    ones_f32 = sb.tile([P, 2, P], F32, name="ones_f32")
    nc.vector.memset(ones_f32[:], 1.0)
    expbias = sb.tile([P, 1], F32, name="expbias")
    nc.vector.memset(expbias[:], EXP_BIAS)
    ones2 = sb.tile([P, 2, P], BF16, name="ones2")

    # --- K/V staging (inputs arrive bf16; DMA straight to matmul tiles) ---
    KTs, Vs = [], []
    for b in range(B):
        KTs.append(sb.tile([P, KL], MMDT, name=f"KT{b}"))
        Vs.append(sb.tile([P, n_kc_total, P], BF16, name=f"V{b}"))

    def emit_qload(b, h, split=1):
        QTt = sb_q.tile([P, QL], MMDT, tag="qtt")
        n = QL // split
        for i in range(split):
            nc.sync.dma_start(QTt[:, i * n:(i + 1) * n],
                              q_ap[b, h][:, i * n:(i + 1) * n])
        return QTt

    G = 4
    gk = KL // G
    gc = n_kc_total // G

    def emit_kv_chunk(b, g, kv="kv"):
        if "k" in kv:
            nc.sync.dma_start(KTs[b][:, g * gk:(g + 1) * gk],
                              k_ap[b][:, g * gk:(g + 1) * gk])
        if "v" in kv:
            nc.sync.dma_start(Vs[b][:, g * gc:(g + 1) * gc, :],
                              v_ap[b][:, g * gc:(g + 1) * gc, :])

    # startup order: the first S-pair needs Q cols [0:512] and K chunks
    # 0-1; issue exactly those two DMAs first, then interleave the rest
    # by when the pipeline needs them.
    QTt0 = sb_q.tile([P, QL], MMDT, tag="qtt")
    nc.sync.dma_start(QTt0[:, :QT], q_ap[0, 0][:, :QT])
    nc.sync.dma_start(KTs[0][:, :2 * KC], k_ap[0][:, :2 * KC])
    nc.vector.tensor_copy(ones2[:], ones_f32[:])
    nc.sync.dma_start(QTt0[:, QT:], q_ap[0, 0][:, QT:])
    nc.sync.dma_start(KTs[0][:, 2 * KC:gk], k_ap[0][:, 2 * KC:gk])
    emit_kv_chunk(0, 0, kv="v")
    qtt_cur = QTt0
    for b in range(B):
        for g in range(G):
            if b == 0 and g == 0:
                continue
            emit_kv_chunk(b, g)

    # --- global pipeline over (item, pair) ---
    heads = [(b, h) for b in range(B) for h in range(HL)]
    items = [(bi, qt) for bi in range(len(heads)) for qt in range(n_qt)]

    # precompute schedule entries: one per pair, tagged with item info
    sched = []
    for it, (bi, qt) in enumerate(items):
        n_kc = (qt + 1) * qt_per_kc
        n_pc = n_kc // 2
        for pc in range(n_pc):
            sched.append((it, bi, qt, pc, n_pc))

    state = {}          # per-item runtime tiles: (O_ps, D_ps, QTt, Obh)
    pts = {}            # pair index in sched -> (PT2, colp, ...)

    def cols_of(kc, qt):
        diag_j = kc - qt * qt_per_kc
        col0 = max(0, diag_j) * KC if diag_j >= 0 else 0
        return col0, col0, diag_j

    def emit_S_pair(si):
        it, bi, qt, pc, n_pc = sched[si]
        b, h = heads[bi]
        O_ps, D_ps, QTt, Obh = state[it]
        q0 = qt * QT
        kc0, kc1 = 2 * pc, 2 * pc + 1
        c0, cm0, dj0 = cols_of(kc0, qt)
        c1, cm1, dj1 = cols_of(kc1, qt)
        colp = c0
        S2 = ps_s.tile([P, 2, QT], F32, tag="s")
        nc.tensor.matmul(
            S2[:, 0, cm0:], KTs[b][:, kc0 * KC:(kc0 + 1) * KC],
            QTt[:, q0 + cm0:q0 + QT], start=True, stop=True,
            skip_group_check=True)
        nc.tensor.matmul(
            S2[:, 1, cm1:], KTs[b][:, kc1 * KC:(kc1 + 1) * KC],
            QTt[:, q0 + cm1:q0 + QT], start=True, stop=True,
            skip_group_check=True)
        PT2 = sb_pt.tile([P, 2, QT], BF16, tag="pt")
        nc.scalar.activation(
            PT2[:, :, colp:], S2[:, :, colp:],
            mybir.ActivationFunctionType.Exp, scale=scale,
            bias=expbias[:])
        # post-exp masking on Pool (off the ACT critical path)
        if dj0 >= 0:
            nc.gpsimd.affine_select(
                out=PT2[:, 0, c0:c0 + P], in_=PT2[:, 0, c0:c0 + P],
                compare_op=mybir.AluOpType.is_ge, fill=0.0,
                base=0, pattern=[[1, P]], channel_multiplier=-1)
        if dj1 >= 0:
            nc.gpsimd.affine_select(
                out=PT2[:, 1, c1:c1 + P], in_=PT2[:, 1, c1:c1 + P],
                compare_op=mybir.AluOpType.is_ge, fill=0.0,
                base=0, pattern=[[1, P]], channel_multiplier=-1)
        pts[si] = (PT2, colp, c0, c1)

    def emit_OD_pair(si):
        it, bi, qt, pc, n_pc = sched[si]
        b, h = heads[bi]
        O_ps, D_ps, QTt, Obh = state[it]
        PT2, colp, c0, c1 = pts.pop(si)
        first, last = pc == 0, pc == n_pc - 1
        nc.tensor.matmul(
            D_ps[:, c0:], ones2[:, 0, :],
            PT2[:, 0, c0:], start=first, stop=False,
            skip_group_check=True)
        nc.tensor.matmul(
            D_ps[:, c1:], ones2[:, 0, :],
            PT2[:, 1, c1:], start=False, stop=last,
            skip_group_check=True)
        nc.tensor.matmul(
            O_ps[:, c0:], Vs[b][:, 2 * pc, :],
            PT2[:, 0, c0:], start=first, stop=False,
            skip_group_check=True)
        nc.tensor.matmul(
            O_ps[:, c1:], Vs[b][:, 2 * pc + 1, :],
            PT2[:, 1, c1:], start=False, stop=last,
            skip_group_check=True)
        if last:
            q0 = qt * QT
            den = sb_o.tile([P, QT], F32, tag="den")  # noqa
            nc.vector.reciprocal(den[:], D_ps[:])
            nc.vector.tensor_mul(Obh[:, q0:q0 + QT], O_ps[:], den[:])
            nc.sync.dma_start(o_ap[b, h][:, q0:q0 + QT],
                              Obh[:, q0:q0 + QT])

    obh0 = sb_o.tile([P, QL], F32, tag="obh")
    cur = {"qtt": qtt_cur, "obh": obh0}
    nxt = {}
    for si, (it, bi, qt, pc, n_pc) in enumerate(sched):
        if pc == 0:
            if qt == 0 and it > 0:
                cur, nxt = nxt, {}
            o_t = ps_o.tile([P, QT], F32, tag="o")
            d_t = ps_d.tile([P, QT], F32, tag="d")
            state[it] = (o_t, d_t, cur["qtt"], cur["obh"])
        emit_S_pair(si)
        # prefetch next head's Q + obh during its qt==0
        if qt == 0 and pc == min(1, n_pc - 1) and bi + 1 < len(heads):
            obh_n = sb_o.tile([P, QL], F32, tag="obh")
            nxt = {"qtt": emit_qload(*heads[bi + 1]), "obh": obh_n}
        if si >= PIPE_PAIRS:
            emit_OD_pair(si - PIPE_PAIRS)
    for si in range(len(sched) - PIPE_PAIRS, len(sched)):
        emit_OD_pair(si)


def build_nc(B=2, QL=2048, KL=2048, HL=4, D=128, reps=1):
    nc = bacc.Bacc("TRN2", target_bir_lowering=False, debug=False,
                   num_devices=8)
    q = nc.dram_tensor("q", [B, HL, D, QL], MMDT, kind="ExternalInput")
    k = nc.dram_tensor("k", [B, D, KL], MMDT, kind="ExternalInput")
    v = nc.dram_tensor("v", [B, P, KL // P, P], MMDT, kind="ExternalInput")
    o = nc.dram_tensor("out", [B, HL, D, QL], F32, kind="ExternalOutput")
    from contextlib import ExitStack
    with tile.TileContext(nc) as tc:
        for _ in range(reps):
            with ExitStack() as ctx:
                emit_attention(nc, tc, ctx, q.ap(), k.ap(), v.ap(), o.ap(),
                               B, QL, KL, HL, D)
    nc.compile()
    return nc


def shard_inputs(q, k, v, n_cores=8):
    B, QL, H, D = q.shape
    KL = k.shape[1]
    HL = H // n_cores
    bf = ml_dtypes.bfloat16
    in_maps = []
    for c in range(n_cores):
        in_maps.append({
            "q": np.ascontiguousarray(
                q[:, :, HL * c:HL * (c + 1), :].transpose(0, 2, 3, 1)
            ).astype(bf),
            "k": np.ascontiguousarray(
                k[:, :, c, :].transpose(0, 2, 1)).astype(bf),
            "v": np.ascontiguousarray(
                v[:, :, c, :].reshape(B, KL // P, P, D).transpose(0, 2, 1, 3)
            ).astype(bf),
        })
    return in_maps


_NC_CACHE = {}


def kernel(q: np.ndarray, k: np.ndarray, v: np.ndarray) -> np.ndarray:
    B, QL, H, D = q.shape
    KL, KVH = k.shape[1], k.shape[2]
    n_cores = 8
    HL = H // n_cores
    assert KVH == n_cores and H == 32 and D == 128

    if "nc" not in _NC_CACHE:
        _NC_CACHE["nc"] = build_nc(B=B, QL=QL, KL=KL, HL=HL, D=D)
    nc = _NC_CACHE["nc"]

    q = np.asarray(q, dtype=np.float32)
    k = np.asarray(k, dtype=np.float32)
    v = np.asarray(v, dtype=np.float32)
    in_maps = shard_inputs(q, k, v, n_cores)
    res = run_bass_kernel_spmd(nc, in_maps, list(range(n_cores)))
    return np.concatenate(
        [r["out"].transpose(0, 3, 1, 2) for r in res.results], axis=2)



# revision 6
# speedup vs baseline: 1.4976x; 1.4976x over previous
"""Causal GQA attention for Trainium2, sharded across 8 NeuronCores.

Problem: q [2, 2048, 32, 128], k/v [2, 2048, 8, 128] fp32, causal, GQA
group = 4. Sharding: core i gets kv-head i plus its 4 q-heads, both
batch elements (tensor-parallel over heads, no collectives); outputs
are concatenated on the head axis.

Per-core kernel (flash-style, transposed-S layout), one global
software pipeline over (head, q-tile, k-chunk-pair):
  - inputs are converted to bf16 on the HOST (halves input DMA bytes
    and removes all on-device dtype-conversion copies).
  - S^T[k,q] = (QK^T)^T per 128-wide k-chunk via f32-PSUM matmuls, with
    the causal column range truncated per chunk.
  - S chunks are emitted as PSUM pair tiles [128,2,512]; ONE ScalarE
    exp per pair writes P^T pairs in bf16, with a constant bias folded
    in (softmax-invariant).
  - causal masking is post-exp on the otherwise-idle GPSIMD engine
    (affine_select triangles), off the exp critical path.
  - O and the denominator come from ONE flipped matmul per
    (k-chunk, q-chunk) block: stationary = P^T chunk [128k, 128q],
    moving = V_ext [128k, 129] where column 128 is all-ones — the
    output [q, 129] accumulates [O | D] in PSUM. This halves the PE
    moving-column count vs separate O^T and denominator matmuls.
  - normalization: DVE reciprocal of the D column + per-partition
    tensor_scalar multiply, stored [q, d]-major straight to HBM.
Flip blocks trail the S/exp stream by PIPE_PAIRS pairs globally, so no
per-tile pipeline drain occurs.
"""

import math

import ml_dtypes
import numpy as np

import concourse.tile as tile
from concourse import bacc, mybir
from concourse.bass_utils import run_bass_kernel_spmd

P = 128
F32 = mybir.dt.float32
MMDT = mybir.dt.bfloat16
BF16 = mybir.dt.bfloat16
PIPE_PAIRS = 2
VW = 132  # V_ext row width: 128 d + 1 ones + 3 pad (4B alignment)
EXP_BIAS = -2.5


def emit_attention(nc, tc, ctx, q_ap, k_ap, v_ap, o_ap, B, QL, KL, HL, D):
    assert D == P
    QT = 512
    KC = P
    n_qt = QL // QT
    n_kc_total = KL // KC
    qt_per_kc = QT // KC
    scale = 1.0 / math.sqrt(D)

    sb = ctx.enter_context(tc.tile_pool(name="sb", bufs=1))
    sb_q = ctx.enter_context(tc.tile_pool(name="sb_q", bufs=2))
    sb_pt = ctx.enter_context(tc.tile_pool(name="sb_pt", bufs=5))
    sb_o = ctx.enter_context(tc.tile_pool(name="sb_o", bufs=4))
    sb_d = ctx.enter_context(tc.tile_pool(name="sb_d", bufs=4))
    ps_s = ctx.enter_context(tc.tile_pool(name="ps_s", bufs=2, space="PSUM"))
    ps_od = ctx.enter_context(tc.tile_pool(name="ps_od", bufs=2, space="PSUM"))

    # --- constants / engine warmups ---
    warm = sb.tile([P, 1], F32, name="warm")
    nc.vector.memset(warm[:], 0.0)
    nc.scalar.activation(warm[:], warm[:],
                         mybir.ActivationFunctionType.Exp)
    poolwarm = sb.tile([P, 8], F32, name="poolwarm")
    nc.gpsimd.memset(poolwarm[:], 0.0)
    expbias = sb.tile([P, 1], F32, name="expbias")
    nc.vector.memset(expbias[:], EXP_BIAS)

    # --- K/V staging (inputs arrive bf16; DMA straight to matmul tiles) ---
    KTs, Vs = [], []
    for b in range(B):
        KTs.append(sb.tile([P, KL], MMDT, name=f"KT{b}"))
        Vs.append(sb.tile([P, n_kc_total, VW], BF16, name=f"V{b}"))

    def emit_qload(b, h, split=1):
        QTt = sb_q.tile([P, QL], MMDT, tag="qtt")
        n = QL // split
        for i in range(split):
            nc.sync.dma_start(QTt[:, i * n:(i + 1) * n],
                              q_ap[b, h][:, i * n:(i + 1) * n])
        return QTt

    G = 4
    gk = KL // G
    gc = n_kc_total // G

    def emit_kv_chunk(b, g, kv="kv"):
        if "k" in kv:
            nc.sync.dma_start(KTs[b][:, g * gk:(g + 1) * gk],
                              k_ap[b][:, g * gk:(g + 1) * gk])
        if "v" in kv:
            nc.sync.dma_start(Vs[b][:, g * gc:(g + 1) * gc, :],
                              v_ap[b][:, g * gc:(g + 1) * gc, :])

    # startup order: the first S-pair needs Q cols [0:512] and K chunks
    # 0-1; issue exactly those two DMAs first, then the rest.
    QTt0 = sb_q.tile([P, QL], MMDT, tag="qtt")
    nc.sync.dma_start(QTt0[:, :QT], q_ap[0, 0][:, :QT])
    nc.sync.dma_start(KTs[0][:, :2 * KC], k_ap[0][:, :2 * KC])
    if QL > QT:
        nc.sync.dma_start(QTt0[:, QT:], q_ap[0, 0][:, QT:])
    if gk > 2 * KC:
        nc.sync.dma_start(KTs[0][:, 2 * KC:gk], k_ap[0][:, 2 * KC:gk])
    emit_kv_chunk(0, 0, kv="v")
    qtt_cur = QTt0
    for b in range(B):
        for g in range(G):
            if b == 0 and g == 0:
                continue
            emit_kv_chunk(b, g)

    # --- global pipeline over (item, pair) ---
    heads = [(b, h) for b in range(B) for h in range(HL)]
    items = [(bi, qt) for bi in range(len(heads)) for qt in range(n_qt)]

    sched = []
    for it, (bi, qt) in enumerate(items):
        n_kc = (qt + 1) * qt_per_kc
        n_pc = n_kc // 2
        for pc in range(n_pc):
            sched.append((it, bi, qt, pc, n_pc))

    state = {}          # per-item: (OD_A, OD_B, QTt)
    pts = {}            # pair index in sched -> PT2

    def emit_S_pair(si):
        it, bi, qt, pc, n_pc = sched[si]
        b, h = heads[bi]
        QTt = state[it][2]
        q0 = qt * QT
        kc0, kc1 = 2 * pc, 2 * pc + 1
        dj0 = kc0 - qt * qt_per_kc
        dj1 = kc1 - qt * qt_per_kc
        c0 = max(0, dj0) * KC
        c1 = max(0, dj1) * KC
        S2 = ps_s.tile([P, 2, QT], F32, tag="s")
        nc.tensor.matmul(
            S2[:, 0, c0:], KTs[b][:, kc0 * KC:(kc0 + 1) * KC],
            QTt[:, q0 + c0:q0 + QT], start=True, stop=True,
            skip_group_check=True)
        nc.tensor.matmul(
            S2[:, 1, c0:], KTs[b][:, kc1 * KC:(kc1 + 1) * KC],
            QTt[:, q0 + c0:q0 + QT], start=True, stop=True,
            skip_group_check=True)
        PT2 = sb_pt.tile([P, 2, QT], BF16, tag="pt")
        nc.scalar.activation(
            PT2[:, :, c0:], S2[:, :, c0:],
            mybir.ActivationFunctionType.Exp, scale=scale,
            bias=expbias[:])
        # post-exp triangle masking on Pool (off the ACT critical path)
        if dj0 >= 0:
            nc.gpsimd.affine_select(
                out=PT2[:, 0, c0:c0 + P], in_=PT2[:, 0, c0:c0 + P],
                compare_op=mybir.AluOpType.is_ge, fill=0.0,
                base=0, pattern=[[1, P]], channel_multiplier=-1)
        if dj1 >= 0:
            nc.gpsimd.affine_select(
                out=PT2[:, 1, c1:c1 + P], in_=PT2[:, 1, c1:c1 + P],
                compare_op=mybir.AluOpType.is_ge, fill=0.0,
                base=0, pattern=[[1, P]], channel_multiplier=-1)
        pts[si] = PT2

    def emit_flip_pair(si):
        it, bi, qt, pc, n_pc = sched[si]
        b, h = heads[bi]
        OD_A, OD_B, QTt = state[it]
        PT2 = pts.pop(si)
        q0 = qt * QT
        for i in range(2):
            kc = 2 * pc + i
            dj = kc - qt * qt_per_kc
            for qc in range(max(dj, 0), 4):
                od = OD_A if qc < 2 else OD_B
                # start=True marks the whole 2KB PSUM zero-region (the
                # full bank, covering BOTH qchunk slots of this od tile)
                # as pending-zero; only the tile's first matmul may set
                # it, or it would wipe the sibling slot's accumulation.
                # The sibling's first write still zero-initializes via
                # that same pending mark.
                first = kc == 0 and qc % 2 == 0
                last = kc == qt * qt_per_kc + qc
                nc.tensor.matmul(
                    od[:, qc % 2, :VW - 3],
                    PT2[:, i, qc * KC:(qc + 1) * KC],
                    Vs[b][:, kc, :VW - 3],
                    start=first, stop=last,
                    skip_group_check=True)
        # qchunks (0,1) finish at pair pc==2*qt; (2,3) at pc==2*qt+1
        fin = None
        if pc == 2 * qt:
            fin, base = OD_A, 0
        elif pc == 2 * qt + 1:
            fin, base = OD_B, 2
        if fin is not None:
            den = sb_d.tile([P, 2], F32, tag="den")
            nc.vector.reciprocal(den[:], fin[:, :, P])
            ob = sb_o.tile([P, 2, P], F32, tag="ob")
            for j in range(2):
                nc.vector.tensor_scalar_mul(
                    ob[:, j, :], fin[:, j, :P], den[:, j:j + 1])
                nc.sync.dma_start(
                    o_ap[b, h][q0 + (base + j) * P:q0 + (base + j + 1) * P, :],
                    ob[:, j, :])

    cur = {"qtt": qtt_cur}
    nxt = {}
    for si, (it, bi, qt, pc, n_pc) in enumerate(sched):
        if pc == 0:
            if qt == 0 and it > 0:
                cur, nxt = nxt, {}
            od_a = ps_od.tile([P, 2, VW], F32, tag="oda")
            od_b = ps_od.tile([P, 2, VW], F32, tag="odb")
            state[it] = (od_a, od_b, cur["qtt"])
        emit_S_pair(si)
        # prefetch next head's Q during its qt==0
        if qt == 0 and pc == min(1, n_pc - 1) and bi + 1 < len(heads):
            nxt = {"qtt": emit_qload(*heads[bi + 1])}
        if si >= PIPE_PAIRS:
            emit_flip_pair(si - PIPE_PAIRS)
    for si in range(len(sched) - PIPE_PAIRS, len(sched)):
        emit_flip_pair(si)


def build_nc(B=2, QL=2048, KL=2048, HL=4, D=128, reps=1):
    nc = bacc.Bacc("TRN2", target_bir_lowering=False, debug=False,
                   num_devices=8)
    q = nc.dram_tensor("q", [B, HL, D, QL], MMDT, kind="ExternalInput")
    k = nc.dram_tensor("k", [B, D, KL], MMDT, kind="ExternalInput")
    v = nc.dram_tensor("v", [B, P, KL // P, VW], MMDT, kind="ExternalInput")
    o = nc.dram_tensor("out", [B, HL, QL, D], F32, kind="ExternalOutput")
    from contextlib import ExitStack
    with tile.TileContext(nc) as tc:
        for _ in range(reps):
            with ExitStack() as ctx:
                emit_attention(nc, tc, ctx, q.ap(), k.ap(), v.ap(), o.ap(),
                               B, QL, KL, HL, D)
    nc.compile()
    return nc


def shard_inputs(q, k, v, n_cores=8):
    B, QL, H, D = q.shape
    KL = k.shape[1]
    HL = H // n_cores
    bf = ml_dtypes.bfloat16
    in_maps = []
    for c in range(n_cores):
        vc = v[:, :, c, :].reshape(B, KL // P, P, D).transpose(0, 2, 1, 3)
        vx = np.zeros((B, P, KL // P, VW), np.float32)
        vx[:, :, :, :D] = vc
        vx[:, :, :, D] = 1.0
        in_maps.append({
            "q": np.ascontiguousarray(
                q[:, :, HL * c:HL * (c + 1), :].transpose(0, 2, 3, 1)
            ).astype(bf),
            "k": np.ascontiguousarray(
                k[:, :, c, :].transpose(0, 2, 1)).astype(bf),
            "v": vx.astype(bf),
        })
    return in_maps


_NC_CACHE = {}


def kernel(q: np.ndarray, k: np.ndarray, v: np.ndarray) -> np.ndarray:
    B, QL, H, D = q.shape
    KL, KVH = k.shape[1], k.shape[2]
    n_cores = 8
    HL = H // n_cores
    assert KVH == n_cores and H == 32 and D == 128

    if "nc" not in _NC_CACHE:
        _NC_CACHE["nc"] = build_nc(B=B, QL=QL, KL=KL, HL=HL, D=D)
    nc = _NC_CACHE["nc"]

    q = np.asarray(q, dtype=np.float32)
    k = np.asarray(k, dtype=np.float32)
    v = np.asarray(v, dtype=np.float32)
    in_maps = shard_inputs(q, k, v, n_cores)
    res = run_bass_kernel_spmd(nc, in_maps, list(range(n_cores)))
    return np.concatenate(
        [r["out"].transpose(0, 2, 1, 3) for r in res.results], axis=2)
